# revision 1
# baseline (speedup 1.0000x reference)
"""DeepSeekV3-style MoE block on 8 Trainium2 NeuronCores.

Strategy (expert-parallel, host-routed dispatch/combine):
  - Host computes the (tiny) sigmoid gate in fp32 numpy, does top-2 selection
    and builds per-expert token lists (the "all-to-all dispatch" happens while
    sharding the inputs).
  - Core e runs expert e's SwiGLU over its gathered tokens (padded to a fixed
    capacity) plus a 1/8 token-slice of the shared expert, all in bf16 on the
    TensorEngine with fp32 PSUM accumulation.  Gate scaling is applied on-chip.
  - The host scatter-adds the per-core outputs back into the full [B,S,H]
    tensor (the "combine" happens while unsharding).

All matmuls are laid out so no on-chip transposes are needed:
  phase A:  act[f,c] = silu(w1[h,f].T @ x[h,c]) * (w3[h,f].T @ x[h,c])
  phase B:  y[c,h]   = act[f,c].T @ w2[f,h]    (scaled by the gate weight)
Host-side pre-tiling puts every DRAM operand in [128, ...] partition-major
layout so each DMA is contiguous.
"""

import hashlib
import os
import sys

for _p in ("/opt/trn_rl_repo", "/opt/pypackages"):
    if _p not in sys.path:
        sys.path.append(_p)

from contextlib import ExitStack

import numpy as np
import ml_dtypes

import concourse.bacc as bacc
import concourse.mybir as mybir
import concourse.tile as tile
from concourse import bass2jax
from concourse.bass_utils import run_bass_kernel_spmd

_NEFF_CACHE_DIR = os.path.expanduser("~/.cache/bass_neff_cache")
_active_build_key = None   # set by _get_nc around the PJRT dispatch


def _install_neff_cache():
    """Persist the compiled bass_exec NEFF across processes.

    The walrus backend takes minutes for this kernel and has no cache of its
    own.  The HLO bytes are not byte-stable across processes (volatile ids /
    debug metadata), so the cache key is derived from the *build inputs*
    (capacities + CFG + build source) instead.  Only the renamed NEFF bytes
    are stored; each request re-wraps them around its own HLO."""
    if getattr(bass2jax, "_ant_neff_cache_wrapped", False):
        return
    inner = bass2jax.neuronx_cc_hook

    captured = {}
    orig_rename = bass2jax.rename_neff_tensors_and_patch_header

    def capture_rename(neff_path, mapping):
        data = orig_rename(neff_path, mapping)
        captured["neff"] = data
        return data

    bass2jax.rename_neff_tensors_and_patch_header = capture_rename

    def cached_hook(code, code_format, platform_version, file_prefix):
        c = code if isinstance(code, (bytes, bytearray)) else str(code).encode()
        if b"bass_exec" not in c or _active_build_key is None:
            return inner(code, code_format, platform_version, file_prefix)
        from libneuronxla.libncc import _wrap_neff_as_custom_call

        path = os.path.join(_NEFF_CACHE_DIR, _active_build_key + ".neff")
        try:
            if os.path.exists(path):
                with open(path, "rb") as f:
                    return 0, _wrap_neff_as_custom_call(bytes(c), f.read())
        except Exception:
            pass
        captured.pop("neff", None)
        r = inner(code, code_format, platform_version, file_prefix)
        neff = captured.pop("neff", None)
        if neff is not None:
            try:
                os.makedirs(_NEFF_CACHE_DIR, exist_ok=True)
                tmp = f"{path}.tmp{os.getpid()}"
                with open(tmp, "wb") as f:
                    f.write(neff)
                os.replace(tmp, path)
            except Exception:
                pass
        return r

    bass2jax.neuronx_cc_hook = cached_hook
    bass2jax._ant_neff_cache_wrapped = True


_install_neff_cache()


def _build_key(C_r, C_s):
    import inspect

    src = inspect.getsource(_build) + inspect.getsource(_chunks)
    blob = f"moe-ep-v1|{C_r}|{C_s}|{sorted(CFG.items())}|{src}"
    return hashlib.sha256(blob.encode()).hexdigest()

BF16 = ml_dtypes.bfloat16
P = 128
H = 2048
F = 1408
E = 8
TOPK = 2
NCORES = 8
KH = H // P   # 16 contraction tiles over H
KF = F // P   # 11 contraction tiles over F
HB = H // 512  # 4 output column blocks

FP32 = mybir.dt.float32
BF16_DT = mybir.dt.bfloat16


def _chunks(C, first=None):
    """Split C into 512-wide chunks (+ remainder).  N=512 matmuls amortize
    the PE sequencer's ~165ns/instruction dispatch cost; narrower chunks go
    sequencer-bound.  `first` optionally shrinks the leading chunk so the
    kernel's first matmuls wait on a smaller x transfer."""
    out = []
    c0 = 0
    if first and first < C:
        out.append((0, first))
        c0 = first
    while c0 < C:
        cb = min(512, C - c0)
        out.append((c0, cb))
        c0 += cb
    return out


CFG = {
    "w13_split": 2,   # dma_starts per w1f/w3f tile
    "w13_split0": 1,  # split for the startup-critical f=0 tiles
    "w13_bufs": 4,
    "x_split": 1,     # dma_starts per x remainder piece
    "x_split0": 1,    # dma_starts per x first-chunk piece
    "w2_split": 1,    # dma_starts per w2 f-slice
    "w2_defer_f": 2,  # emit the w2 bulk load at this f iteration
    "out_split": 1,   # dma_starts per output tile
    "ps1_bufs": 2,
    "ps2_bufs": 3,
    "o_bufs": 6,
    "silu_bufs": 3,
    "dma_eng": "sync",  # w13 weight stream issue engine
    "x_eng": "sync",    # x load issue engine
    "w2_eng": "sync",   # bulk w2 load issue engine
    "out_eng": "sync",  # output store issue engine
    "out_bf16": False,  # store outputs as bf16 (halves output DMA + tail)
    "fastboot": False,  # emit first-needed 128-col slices as separate DMAs
    "chunk0": None,     # optional smaller leading chunk (startup latency)
    "warmup_mms": 20,   # dummy matmuls at t=0: warm the PE clock (HAM) while
                        # the first real DMAs are still in flight
}


def _split_dma(eng, dst, src, n):
    w = dst.shape[-1]
    step = -(-w // n)
    for i in range(0, w, step):
        j = min(w, i + step)
        eng.dma_start(dst[:, i:j], src[:, i:j])


def _build(nc, C_r, C_s):
    """Emit the per-core program: routed expert (C_r tokens, gated) then the
    shared-expert slice (C_s tokens)."""
    dram = {}
    for name, shape, dt in [
        ("xr", [P, KH * C_r], BF16_DT),
        ("gr", [P, -(-C_r // P)], FP32),
        ("w1", [P, KF * KH * P], BF16_DT),
        ("w3", [P, KF * KH * P], BF16_DT),
        ("w2", [P, KF * H], BF16_DT),
        ("xs", [P, KH * C_s], BF16_DT),
        ("s1", [P, KF * KH * P], BF16_DT),
        ("s3", [P, KF * KH * P], BF16_DT),
        ("s2", [P, KF * H], BF16_DT),
    ]:
        dram[name] = nc.dram_tensor(name, shape, dt, kind="ExternalInput")
    out_dt = BF16_DT if CFG["out_bf16"] else FP32
    yr = nc.dram_tensor("yr", [C_r, H], out_dt, kind="ExternalOutput")
    ys = nc.dram_tensor("ys", [C_s, H], out_dt, kind="ExternalOutput")

    with tile.TileContext(nc) as tc, ExitStack() as ctx:
        pool = ctx.enter_context(tc.tile_pool(name="main", bufs=1))
        psum = ctx.enter_context(tc.tile_pool(name="ps", bufs=1, space="PSUM"))
        dmae = getattr(nc, CFG["dma_eng"])
        xeng = getattr(nc, CFG["x_eng"])
        w2eng = getattr(nc, CFG["w2_eng"])
        oeng = getattr(nc, CFG["out_eng"])

        if CFG["warmup_mms"]:
            # No DMA dependency: memset SBUF, then back-to-back matmuls so the
            # PE HAM/p-state is warm by the time the first weights arrive.
            wz = pool.tile([P, P], BF16_DT, tag="warm_w", bufs=1)
            rz = pool.tile([P, 512], BF16_DT, tag="warm_r", bufs=1)
            nc.gpsimd.memset(wz[:], 0.0)
            nc.gpsimd.memset(rz[:], 0.0)
            pz = psum.tile([P, 512], FP32, tag="warm_ps", bufs=1)
            for _ in range(CFG["warmup_mms"]):
                nc.tensor.matmul(pz[:], lhsT=wz[:], rhs=rz[:], start=True,
                                 stop=True)

        def problem(tag, xd, w1d, w3d, w2d, yd, C, gd=None):
            # resident x: [128, KH*C]; DMA per (chunk, contraction-tile) so the
            # first chunk's columns land before anything else
            x_sb = pool.tile([P, KH * C], BF16_DT, tag=f"x_{tag}", bufs=1)
            g_sb = None
            if gd is not None:
                g_sb = pool.tile([P, -(-C // P)], FP32, tag=f"g_{tag}", bufs=1)
                nc.sync.dma_start(g_sb[:], gd[:])

            w2_sb = pool.tile([P, KF * H], BF16_DT, tag="w2", bufs=1)
            act_sb = pool.tile([P, KF * C], BF16_DT, tag=f"act_{tag}", bufs=1)

            # ---- phase A: act[f, c] = silu(x@w1.T) * (x@w3.T), [F, C] layout
            chunks = _chunks(C, first=CFG["chunk0"] if gd is not None else None)
            for f in range(KF):
                wsplit = CFG["w13_split0"] if f == 0 else CFG["w13_split"]
                w1f = pool.tile([P, KH * P], BF16_DT, tag="w1f", bufs=CFG["w13_bufs"])
                w3f = pool.tile([P, KH * P], BF16_DT, tag="w3f", bufs=CFG["w13_bufs"])
                if f == 0 and CFG["fastboot"]:
                    # land the first accumulation step's 128 columns ASAP so
                    # the very first matmul isn't gated on a 512KB transfer
                    for wt, wdr in ((w1f, w1d), (w3f, w3d)):
                        dmae.dma_start(wt[:, :P], wdr[:, :P])
                        _split_dma(dmae, wt[:, P:], wdr[:, P : KH * P], wsplit)
                else:
                    _split_dma(
                        dmae, w1f[:], w1d[:, f * KH * P : (f + 1) * KH * P], wsplit
                    )
                    _split_dma(
                        dmae, w3f[:], w3d[:, f * KH * P : (f + 1) * KH * P], wsplit
                    )
                if f == CFG["w2_defer_f"]:
                    # defer the (large, phase-B-only) w2 load past startup
                    for ff in range(KF):
                        _split_dma(
                            w2eng,
                            w2_sb[:, ff * H : (ff + 1) * H],
                            w2d[:, ff * H : (ff + 1) * H],
                            CFG["w2_split"],
                        )
                for ci, (c0, cb) in enumerate(chunks):
                    if f == 0:
                        if ci == 0:
                            # startup-critical: first chunk's columns, finely split
                            for kk in range(KH):
                                _split_dma(
                                    xeng,
                                    x_sb[:, kk * C + c0 : kk * C + c0 + cb],
                                    xd[:, kk * C + c0 : kk * C + c0 + cb],
                                    CFG["x_split0"],
                                )
                        elif ci == 1:
                            # everything else in one go, ahead of the w2 bulk
                            for kk in range(KH):
                                _split_dma(
                                    xeng,
                                    x_sb[:, kk * C + c0 : kk * C + C],
                                    xd[:, kk * C + c0 : kk * C + C],
                                    CFG["x_split"],
                                )
                    ps1 = psum.tile([P, cb], FP32, tag="ps1", bufs=CFG["ps1_bufs"])
                    ps3 = psum.tile([P, cb], FP32, tag="ps3", bufs=CFG["ps1_bufs"])
                    for kk in range(KH):
                        nc.tensor.matmul(
                            ps1[:],
                            lhsT=w1f[:, kk * P : (kk + 1) * P],
                            rhs=x_sb[:, kk * C + c0 : kk * C + c0 + cb],
                            start=(kk == 0),
                            stop=(kk == KH - 1),
                        )
                    for kk in range(KH):
                        nc.tensor.matmul(
                            ps3[:],
                            lhsT=w3f[:, kk * P : (kk + 1) * P],
                            rhs=x_sb[:, kk * C + c0 : kk * C + c0 + cb],
                            start=(kk == 0),
                            stop=(kk == KH - 1),
                        )
                    tmp = pool.tile([P, cb], BF16_DT, tag="silu", bufs=CFG["silu_bufs"])
                    nc.scalar.activation(
                        tmp[:], ps1[:], mybir.ActivationFunctionType.Silu
                    )
                    nc.vector.tensor_mul(
                        act_sb[:, f * C + c0 : f * C + c0 + cb], tmp[:], ps3[:]
                    )

            # ---- phase B: y[c, h] = act.T @ w2, gate-scaled
            for ct in range(-(-C // P)):
                tp = min(P, C - ct * P)   # partial final token-tile
                for hb in range(HB):
                    ps2 = psum.tile([P, 512], FP32, tag="ps2", bufs=CFG["ps2_bufs"])
                    for f in range(KF):
                        nc.tensor.matmul(
                            ps2[:tp],
                            lhsT=act_sb[:, f * C + ct * P : f * C + ct * P + tp],
                            rhs=w2_sb[:, f * H + hb * 512 : f * H + (hb + 1) * 512],
                            start=(f == 0),
                            stop=(f == KF - 1),
                        )
                    o = pool.tile([P, 512], out_dt, tag="o", bufs=CFG["o_bufs"])
                    if g_sb is not None:
                        nc.vector.tensor_scalar_mul(
                            o[:tp], ps2[:tp], g_sb[:tp, ct : ct + 1]
                        )
                    else:
                        nc.vector.tensor_copy(o[:tp], ps2[:tp])
                    _split_dma(
                        oeng,
                        yd[ct * P : ct * P + tp, hb * 512 : (hb + 1) * 512],
                        o[:tp],
                        CFG["out_split"],
                    )

        problem("r", dram["xr"].ap(), dram["w1"].ap(), dram["w3"].ap(),
                dram["w2"].ap(), yr.ap(), C_r, gd=dram["gr"].ap())
        problem("s", dram["xs"].ap(), dram["s1"].ap(), dram["s3"].ap(),
                dram["s2"].ap(), ys.ap(), C_s)

    return nc


_cache = {}


def _get_nc(C_r, C_s):
    key = (C_r, C_s, tuple(sorted(CFG.items())))
    if key not in _cache:
        nc = bacc.Bacc("TRN2", target_bir_lowering=False, debug=False,
                       num_devices=NCORES)
        _build(nc, C_r, C_s)
        nc.compile()
        _cache[key] = nc
    return _cache[key]


def _tile_w13(w):
    """[F, H] fp32 -> [128, KF*KH*128] bf16, (f, kk, j) column order."""
    a = np.ascontiguousarray(w, np.float32).astype(BF16)
    return np.ascontiguousarray(
        a.reshape(KF, P, KH, P).transpose(3, 0, 2, 1)
    ).reshape(P, KF * KH * P)


def _tile_w2(w):
    """[H, F] fp32 -> [128, KF*H] bf16, (f, h) column order."""
    a = np.ascontiguousarray(w, np.float32).astype(BF16)
    return np.ascontiguousarray(a.reshape(H, KF, P).transpose(2, 1, 0)).reshape(
        P, KF * H
    )


def _pad_rows(x, n):
    if x.shape[0] == n:
        return x
    out = np.zeros((n, x.shape[1]), x.dtype)
    out[: x.shape[0]] = x
    return out


def _tile_x(x):
    """[C, H] fp32 -> [128, KH*C] bf16, (kk, c) column order."""
    C = x.shape[0]
    a = x.astype(BF16)
    return np.ascontiguousarray(a.reshape(C, KH, P).transpose(2, 1, 0)).reshape(
        P, KH * C
    )


def kernel(hidden_states, gate_w, bias, ws1, ws2, ws3, we1, we2, we3):
    orig_shape = hidden_states.shape
    x = np.ascontiguousarray(
        np.asarray(hidden_states, np.float32).reshape(-1, orig_shape[-1])
    )
    T = x.shape[0]
    gate_w = np.asarray(gate_w, np.float32)
    bias = np.asarray(bias, np.float32)
    we1 = np.asarray(we1, np.float32)
    we2 = np.asarray(we2, np.float32)
    we3 = np.asarray(we3, np.float32)
    assert gate_w.shape[0] == E and we1.shape[0] == E and x.shape[1] == H

    # ---- host router (fp32, matches the reference's selection math)
    logits = x @ gate_w.T                                 # [T, E]
    scores = np.where(
        logits >= 0,
        1.0 / (1.0 + np.exp(-np.abs(logits))),
        1.0 - 1.0 / (1.0 + np.exp(-np.abs(logits))),
    ).astype(np.float32)
    routing = scores + bias[None, :]
    topk = np.argsort(-routing, axis=1, kind="stable")[:, :TOPK]  # [T, K]
    sel = np.take_along_axis(scores, topk, axis=1)
    gates = sel / sel.sum(axis=1, keepdims=True)          # [T, K]

    idx_e = []      # token ids routed to expert e
    gate_e = []     # matching combine weights
    for e in range(E):
        mask = topk == e                      # [T, K], at most one True per row
        rows = np.nonzero(mask.any(axis=1))[0]
        idx_e.append(rows)
        gate_e.append(gates[mask].astype(np.float32))  # row-major -> rows order

    max_n = max(len(r) for r in idx_e)
    C_r = max(64, -(-max_n // 64) * 64)   # routed capacity, multiple of 64
    C_s = max(64, -(-T // (NCORES * 64)) * 64)  # shared tokens per core

    nc = _get_nc(C_r, C_s)

    # ---- build per-core input maps
    shared_w = {
        "s1": _tile_w13(ws1),
        "s3": _tile_w13(ws3),
        "s2": _tile_w2(ws2),
    }
    in_maps = []
    for e in range(E):
        rows = idx_e[e]
        xg = np.zeros((C_r, H), np.float32)
        xg[: len(rows)] = x[rows]
        ctiles = -(-C_r // P)
        g = np.zeros((ctiles * P,), np.float32)
        g[: len(rows)] = gate_e[e]
        m = {
            "xr": _tile_x(xg),
            "gr": np.ascontiguousarray(g.reshape(ctiles, P).T),
            "w1": _tile_w13(we1[e]),
            "w3": _tile_w13(we3[e]),
            "w2": _tile_w2(we2[e]),
            "xs": _tile_x(_pad_rows(x[e * C_s : (e + 1) * C_s], C_s)),
        }
        m.update(shared_w)
        in_maps.append(m)

    global _active_build_key
    _active_build_key = _build_key(C_r, C_s)
    try:
        res = run_bass_kernel_spmd(nc, in_maps, list(range(NCORES))).results
    finally:
        _active_build_key = None

    # ---- host combine
    out = np.zeros((T, H), np.float32)
    for e in range(E):
        rows = idx_e[e]
        out[rows] += res[e]["yr"][: len(rows)]
        lo = e * C_s
        hi = min(T, (e + 1) * C_s)
        if lo < hi:
            out[lo:hi] += res[e]["ys"][: hi - lo]
    return out.reshape(orig_shape).astype(np.float32)



# revision 3
# speedup vs baseline: 1.1670x; 1.1670x over previous
"""DeepSeekV3-style MoE block on 8 Trainium2 NeuronCores.

Strategy (expert-parallel, host-routed dispatch/combine, fp8 DoubleRow math):
  - Host computes the (tiny) sigmoid gate in fp32 numpy, does top-2 selection
    and builds per-expert token lists (the "all-to-all dispatch" happens while
    sharding the inputs).
  - Core e runs expert e's SwiGLU over its gathered tokens (padded to a fixed
    capacity) plus a 1/8 token-slice of the shared expert.  All matmuls run as
    fp8(e4m3) DoubleRow (256-deep contraction, double-pumped PE) with fp32
    PSUM accumulation.  To stay well inside the 2e-2 tolerance each logical
    matmul is a 3-term hi/lo decomposition:
        y = x_hi@w_hi + x_hi@w_lo + x_lo@w_hi
    where *_hi = fp8(v*scale), *_lo = fp8 residual at the same scale.  Weight
    scales (x256 / x16) keep every fp8 operand out of the subnormal range, so
    the scheme is robust even to FTZ hardware.  Measured end-to-end rel err
    ~1.8e-3 (better than an all-bf16 version's 4.1e-3).
  - Gate scaling is applied on-chip via the output copy (gates pre-divided by
    the global 1/4096 fp8 scale on the host); the host scatter-adds per-core
    outputs into the full [B,S,H] tensor (the "combine").

Layouts (all host-pretiled so every DMA is contiguous):
  phase A:  ps1[f,c] = sum_k w1*[h2,f].T @ x*[h2,c]   (DR pairs over KH=16)
            act = silu(ps1/256) * ps3  -> split to fp8 hi/lo at x16 scale
  phase B:  y[c,h]   = sum_f act*[f2,c].T @ w2*[f2,h] (DR pairs over KF2=12,
            f-tile 12 zero-padded)
"""

import hashlib
import os
import sys

for _p in ("/opt/trn_rl_repo", "/opt/pypackages"):
    if _p not in sys.path:
        sys.path.append(_p)

from contextlib import ExitStack

import numpy as np
import ml_dtypes

import concourse.bacc as bacc
import concourse.mybir as mybir
import concourse.tile as tile
from concourse import bass2jax
from concourse.bass_utils import run_bass_kernel_spmd

_NEFF_CACHE_DIR = os.path.expanduser("~/.cache/bass_neff_cache")
_active_build_key = None   # set by _get_nc around the PJRT dispatch


def _install_neff_cache():
    """Persist the compiled bass_exec NEFF across processes.

    The walrus backend takes minutes for this kernel and has no cache of its
    own.  The HLO bytes are not byte-stable across processes (volatile ids /
    debug metadata), so the cache key is derived from the *build inputs*
    (capacities + CFG + build source) instead.  Only the renamed NEFF bytes
    are stored; each request re-wraps them around its own HLO."""
    if getattr(bass2jax, "_ant_neff_cache_wrapped", False):
        return
    inner = bass2jax.neuronx_cc_hook

    captured = {}
    orig_rename = bass2jax.rename_neff_tensors_and_patch_header

    def capture_rename(neff_path, mapping):
        data = orig_rename(neff_path, mapping)
        captured["neff"] = data
        return data

    bass2jax.rename_neff_tensors_and_patch_header = capture_rename

    def cached_hook(code, code_format, platform_version, file_prefix):
        c = code if isinstance(code, (bytes, bytearray)) else str(code).encode()
        if b"bass_exec" not in c or _active_build_key is None:
            return inner(code, code_format, platform_version, file_prefix)
        from libneuronxla.libncc import _wrap_neff_as_custom_call

        path = os.path.join(_NEFF_CACHE_DIR, _active_build_key + ".neff")
        try:
            if os.path.exists(path):
                with open(path, "rb") as f:
                    return 0, _wrap_neff_as_custom_call(bytes(c), f.read())
        except Exception:
            pass
        captured.pop("neff", None)
        r = inner(code, code_format, platform_version, file_prefix)
        neff = captured.pop("neff", None)
        if neff is not None:
            try:
                os.makedirs(_NEFF_CACHE_DIR, exist_ok=True)
                tmp = f"{path}.tmp{os.getpid()}"
                with open(tmp, "wb") as f:
                    f.write(neff)
                os.replace(tmp, path)
            except Exception:
                pass
        return r

    bass2jax.neuronx_cc_hook = cached_hook
    bass2jax._ant_neff_cache_wrapped = True


_install_neff_cache()


def _build_key(C_r, C_s):
    import inspect

    src = inspect.getsource(_build) + inspect.getsource(_chunks)
    blob = f"moe-ep-fp8dr-v1|{C_r}|{C_s}|{sorted(CFG.items())}|{src}"
    return hashlib.sha256(blob.encode()).hexdigest()


E4NP = ml_dtypes.float8_e4m3fn
BF16 = ml_dtypes.bfloat16
P = 128
H = 2048
F = 1408
E = 8
TOPK = 2
NCORES = 8
KH = H // P       # 16 contraction tiles over H (8 DoubleRow pairs)
KF = F // P       # 11 f tiles
KF2 = KF + 1      # f tiles padded to an even count for DR pairing in phase B
HB = H // 512     # 4 output column blocks

FP32 = mybir.dt.float32
BF16_DT = mybir.dt.bfloat16
F8 = mybir.dt.float8e4
DR = mybir.MatmulPerfMode.DoubleRow
SILU = mybir.ActivationFunctionType.Silu
COPY = mybir.ActivationFunctionType.Copy
MULT = mybir.AluOpType.mult
SUBTRACT = mybir.AluOpType.subtract

W13_NAMES = ("w1h", "w1l", "w1m", "w3h", "w3l", "w3m")


def _chunks(C, first=None):
    """Split C into 512-wide chunks (+ remainder).  Wide chunks amortize the
    PE sequencer dispatch; `first` optionally shrinks the leading chunk so the
    kernel's first matmuls wait on a smaller x transfer."""
    out = []
    c0 = 0
    if first and first < C:
        out.append((0, first))
        c0 = first
    while c0 < C:
        cb = min(512, C - c0)
        out.append((c0, cb))
        c0 += cb
    return out


CFG = {
    "w13_bufs": 3,
    "w2_defer_f": 2,  # emit the w2 bulk load at this f iteration
    "ps_bufs": 2,
    "ps2_bufs": 3,
    "o_bufs": 4,
    "silu_bufs": 3,
    "act32_bufs": 3,
    "dma_eng": "sync",    # w13 weight stream issue engine
    "x_eng": "scalar",    # x load issue engine
    "w2_eng": "gpsimd",   # bulk w2 load issue engine
    "out_eng": "gpsimd",  # output store issue engine
    "hi_eng": "vector",   # engine computing the act_hi fp8 cast
    "chunk0": None,       # optional smaller leading chunk (startup latency)
    "warmup_mms": 20,     # dummy matmuls at t=0: warm the PE clock (HAM) while
                          # the first real DMAs are still in flight
}


def _build(nc, C_r, C_s):
    """Emit the per-core program: routed expert (C_r tokens, gated) then the
    shared-expert slice (C_s tokens, 'gate' = 1/4096 output scale)."""
    dram = {}
    for t, C in (("r", C_r), ("s", C_s)):
        ct = -(-C // P)
        specs = [
            (f"xh_{t}", [P, KH, C], F8),
            (f"xl_{t}", [P, KH, C], F8),
            (f"g_{t}", [P, ct], FP32),
            (f"w2h_{t}", [P, KF2, H], F8),
            (f"w2l_{t}", [P, KF2, H], F8),
        ]
        for nm in W13_NAMES:
            specs.append((f"{nm}_{t}", [P, KF, KH, P], F8))
        for name, shape, dt in specs:
            dram[name] = nc.dram_tensor(name, shape, dt, kind="ExternalInput")
        dram[f"y_{t}"] = nc.dram_tensor(f"y_{t}", [C, H], FP32,
                                        kind="ExternalOutput")

    with tile.TileContext(nc) as tc, ExitStack() as ctx:
        pool = ctx.enter_context(tc.tile_pool(name="main", bufs=1))
        psum = ctx.enter_context(tc.tile_pool(name="ps", bufs=1, space="PSUM"))
        dmae = getattr(nc, CFG["dma_eng"])
        xeng = getattr(nc, CFG["x_eng"])
        w2eng = getattr(nc, CFG["w2_eng"])
        oeng = getattr(nc, CFG["out_eng"])

        if CFG["warmup_mms"]:
            # No DMA dependency: memset SBUF, then back-to-back matmuls so the
            # PE p-state is warm by the time the first weights arrive.
            wz = pool.tile([P, P], BF16_DT, tag="warm_w", bufs=1)
            rz = pool.tile([P, 512], BF16_DT, tag="warm_r", bufs=1)
            nc.gpsimd.memset(wz[:], 0.0)
            nc.gpsimd.memset(rz[:], 0.0)
            pz = psum.tile([P, 512], FP32, tag="warm_ps", bufs=1)
            for _ in range(CFG["warmup_mms"]):
                nc.tensor.matmul(pz[:], lhsT=wz[:], rhs=rz[:], start=True,
                                 stop=True)

        # act tiles sized for the larger (routed) problem, reused by the
        # shared problem; f-tile KF2-1 stays zero forever (phase-B DR padding).
        act_h = pool.tile([P, KF2, C_r], F8, tag="act_h", bufs=1)
        act_l = pool.tile([P, KF2, C_r], F8, tag="act_l", bufs=1)
        nc.gpsimd.memset(act_h[:, KF2 - 1, :], 0.0)
        nc.gpsimd.memset(act_l[:, KF2 - 1, :], 0.0)
        # w2 tiles shared across the two problems (bufs=1 -> WAR dependency
        # naturally delays the shared-expert load until routed phase B done).
        w2h_sb = pool.tile([P, KF2, H], F8, tag="w2h", bufs=1)
        w2l_sb = pool.tile([P, KF2, H], F8, tag="w2l", bufs=1)

        def problem(t, C):
            ctiles = -(-C // P)
            x_h = pool.tile([P, KH, C], F8, tag=f"xh_{t}", bufs=1)
            x_l = pool.tile([P, KH, C], F8, tag=f"xl_{t}", bufs=1)
            g_sb = pool.tile([P, ctiles], FP32, tag=f"g_{t}", bufs=1)
            nc.sync.dma_start(g_sb[:], dram[f"g_{t}"].ap())
            yd = dram[f"y_{t}"].ap()

            chunks = _chunks(C, first=CFG["chunk0"] if t == "r" else None)

            # ---- phase A: act[f, c] = silu(h1) * h3 in [F, C] layout,
            #      split to fp8 hi/lo at x16 scale
            for f in range(KF):
                wts = {}
                for nm in W13_NAMES:
                    wt = pool.tile([P, KH, P], F8, tag=nm,
                                   bufs=CFG["w13_bufs"])
                    dmae.dma_start(wt[:], dram[f"{nm}_{t}"].ap()[:, f])
                    wts[nm] = wt
                if f == CFG["w2_defer_f"]:
                    # defer the (large, phase-B-only) w2 load past startup
                    for sbt, dn in ((w2h_sb, f"w2h_{t}"), (w2l_sb, f"w2l_{t}")):
                        for ff in range(KF2):
                            w2eng.dma_start(sbt[:, ff], dram[dn].ap()[:, ff])
                for ci, (c0, cb) in enumerate(chunks):
                    if f == 0:
                        xeng.dma_start(x_h[:, :, c0:c0 + cb],
                                       dram[f"xh_{t}"].ap()[:, :, c0:c0 + cb])
                        xeng.dma_start(x_l[:, :, c0:c0 + cb],
                                       dram[f"xl_{t}"].ap()[:, :, c0:c0 + cb])
                    ps1 = psum.tile([P, cb], FP32, tag="ps1",
                                    bufs=CFG["ps_bufs"])
                    ps3 = psum.tile([P, cb], FP32, tag="ps3",
                                    bufs=CFG["ps_bufs"])
                    for ps, names in ((ps1, W13_NAMES[:3]),
                                      (ps3, W13_NAMES[3:])):
                        terms = ((wts[names[0]], x_h), (wts[names[1]], x_h),
                                 (wts[names[2]], x_l))
                        n = 0
                        for wt, xt in terms:
                            for k in range(KH // 2):
                                nc.tensor.matmul(
                                    ps[:],
                                    lhsT=wt[:, 2 * k:2 * k + 2, :],
                                    rhs=xt[:, 2 * k:2 * k + 2, c0:c0 + cb],
                                    start=(n == 0),
                                    stop=(n == 3 * (KH // 2) - 1),
                                    perf_mode=DR,
                                )
                                n += 1
                    tmp = pool.tile([P, cb], FP32, tag="silu",
                                    bufs=CFG["silu_bufs"])
                    nc.scalar.activation(tmp[:], ps1[:], SILU, scale=1.0 / 256)
                    act32 = pool.tile([P, cb], FP32, tag="act32",
                                      bufs=CFG["act32_bufs"])
                    # act32 = (tmp * 1/16) * ps3  == 16 * act
                    nc.vector.scalar_tensor_tensor(
                        act32[:], tmp[:], 1.0 / 16, ps3[:], MULT, MULT)
                    ah = act_h[:, f, c0:c0 + cb]
                    if CFG["hi_eng"] == "scalar":
                        nc.scalar.activation(ah, act32[:], COPY)
                    else:
                        nc.vector.tensor_copy(ah, act32[:])
                    nc.vector.tensor_sub(act_l[:, f, c0:c0 + cb], act32[:], ah)

            # ---- phase B: y[c, h] = act.T @ w2 over KF2 DR pairs, gate-scaled
            for ct in range(ctiles):
                tp = min(P, C - ct * P)   # partial final token-tile
                for hb in range(HB):
                    ps2 = psum.tile([P, 512], FP32, tag="ps2",
                                    bufs=CFG["ps2_bufs"])
                    n = 0
                    for a_t, w_t in ((act_h, w2h_sb), (act_h, w2l_sb),
                                     (act_l, w2h_sb)):
                        for k in range(KF2 // 2):
                            nc.tensor.matmul(
                                ps2[:tp],
                                lhsT=a_t[:, 2 * k:2 * k + 2,
                                         ct * P:ct * P + tp],
                                rhs=w_t[:, 2 * k:2 * k + 2,
                                        hb * 512:(hb + 1) * 512],
                                start=(n == 0),
                                stop=(n == 3 * (KF2 // 2) - 1),
                                perf_mode=DR,
                            )
                            n += 1
                    o = pool.tile([P, 512], FP32, tag="o", bufs=CFG["o_bufs"])
                    nc.vector.tensor_scalar_mul(
                        o[:tp], ps2[:tp], g_sb[:tp, ct:ct + 1])
                    oeng.dma_start(
                        yd[ct * P:ct * P + tp, hb * 512:(hb + 1) * 512],
                        o[:tp])

        problem("r", C_r)
        problem("s", C_s)

    return nc


_cache = {}


def _get_nc(C_r, C_s):
    key = (C_r, C_s, tuple(sorted(CFG.items())))
    if key not in _cache:
        nc = bacc.Bacc("TRN2", target_bir_lowering=False, debug=False,
                       num_devices=NCORES)
        _build(nc, C_r, C_s)
        nc.compile()
        _cache[key] = nc
    return _cache[key]


# ---------------- host-side tiling / quantization ----------------

def _q8(a):
    return np.asarray(a, np.float32).astype(E4NP)


def _hilo(w, scale):
    """fp8 hi + residual-lo at a common scale."""
    s = np.asarray(w, np.float32) * scale
    hi = s.astype(E4NP)
    lo = (s - hi.astype(np.float32)).astype(E4NP)
    return hi, lo


def _lay_w13(a):
    """[F, H] fp8 -> [P(h_in), KF, KH, P(f_col)]."""
    return np.ascontiguousarray(a.reshape(KF, P, KH, P).transpose(3, 0, 2, 1))


def _tile_w13(w):
    """[F, H] fp32 -> (hi x256, lo x256, mid x16) in lhsT layout."""
    hi, lo = _hilo(w, 256.0)
    m = _q8(np.asarray(w, np.float32) * 16.0)
    return _lay_w13(hi), _lay_w13(lo), _lay_w13(m)


def _lay_w2(a):
    """[H, F] fp8 -> [P(f_in), KF2, H] with f-tile KF2-1 zero."""
    r = a.reshape(H, KF, P).transpose(2, 1, 0)
    out = np.zeros((P, KF2, H), E4NP)
    out[:, :KF] = r
    return np.ascontiguousarray(out)


def _tile_w2(w):
    hi, lo = _hilo(w, 256.0)
    return _lay_w2(hi), _lay_w2(lo)


def _lay_x(a):
    """[C, H] fp8 -> [P(h_in), KH, C]."""
    C = a.shape[0]
    return np.ascontiguousarray(a.reshape(C, KH, P).transpose(2, 1, 0))


def _tile_x(x):
    """[C, H] fp32 -> (x_hi, 16*x_lo) in rhs layout."""
    x = np.asarray(x, np.float32)
    hi = x.astype(E4NP)
    lo = ((x - hi.astype(np.float32)) * 16.0).astype(E4NP)
    return _lay_x(hi), _lay_x(lo)


def _pad_rows(x, n):
    if x.shape[0] == n:
        return x
    out = np.zeros((n, x.shape[1]), x.dtype)
    out[: x.shape[0]] = x
    return out


OUT_SCALE = 1.0 / 4096.0   # undo the x16 act scale and x256 w2 scale


def kernel(hidden_states, gate_w, bias, ws1, ws2, ws3, we1, we2, we3):
    orig_shape = hidden_states.shape
    x = np.ascontiguousarray(
        np.asarray(hidden_states, np.float32).reshape(-1, orig_shape[-1])
    )
    T = x.shape[0]
    gate_w = np.asarray(gate_w, np.float32)
    bias = np.asarray(bias, np.float32)
    we1 = np.asarray(we1, np.float32)
    we2 = np.asarray(we2, np.float32)
    we3 = np.asarray(we3, np.float32)
    assert gate_w.shape[0] == E and we1.shape[0] == E and x.shape[1] == H

    # ---- host router (fp32, matches the reference's selection math)
    logits = x @ gate_w.T                                 # [T, E]
    scores = np.where(
        logits >= 0,
        1.0 / (1.0 + np.exp(-np.abs(logits))),
        1.0 - 1.0 / (1.0 + np.exp(-np.abs(logits))),
    ).astype(np.float32)
    routing = scores + bias[None, :]
    topk = np.argsort(-routing, axis=1, kind="stable")[:, :TOPK]  # [T, K]
    sel = np.take_along_axis(scores, topk, axis=1)
    gates = sel / sel.sum(axis=1, keepdims=True)          # [T, K]

    idx_e = []      # token ids routed to expert e
    gate_e = []     # matching combine weights
    for e in range(E):
        mask = topk == e                      # [T, K], at most one True per row
        rows = np.nonzero(mask.any(axis=1))[0]
        idx_e.append(rows)
        gate_e.append(gates[mask].astype(np.float32))  # row-major -> rows order

    max_n = max(len(r) for r in idx_e)
    C_r = max(64, -(-max_n // 64) * 64)   # routed capacity, multiple of 64
    C_s = max(64, -(-T // (NCORES * 64)) * 64)  # shared tokens per core

    nc = _get_nc(C_r, C_s)

    # ---- build per-core input maps
    shared_w = {}
    for nm, arr in zip(W13_NAMES[:3], _tile_w13(ws1)):
        shared_w[f"{nm}_s"] = arr
    for nm, arr in zip(W13_NAMES[3:], _tile_w13(ws3)):
        shared_w[f"{nm}_s"] = arr
    for nm, arr in zip(("w2h_s", "w2l_s"), _tile_w2(ws2)):
        shared_w[nm] = arr
    ct_s = -(-C_s // P)
    shared_w["g_s"] = np.full((P, ct_s), OUT_SCALE, np.float32)

    in_maps = []
    for e in range(E):
        rows = idx_e[e]
        xg = np.zeros((C_r, H), np.float32)
        xg[: len(rows)] = x[rows]
        ctiles = -(-C_r // P)
        g = np.zeros((ctiles * P,), np.float32)
        g[: len(rows)] = gate_e[e] * OUT_SCALE
        m = {"g_r": np.ascontiguousarray(g.reshape(ctiles, P).T)}
        m["xh_r"], m["xl_r"] = _tile_x(xg)
        for nm, arr in zip(W13_NAMES[:3], _tile_w13(we1[e])):
            m[f"{nm}_r"] = arr
        for nm, arr in zip(W13_NAMES[3:], _tile_w13(we3[e])):
            m[f"{nm}_r"] = arr
        for nm, arr in zip(("w2h_r", "w2l_r"), _tile_w2(we2[e])):
            m[nm] = arr
        m["xh_s"], m["xl_s"] = _tile_x(
            _pad_rows(x[e * C_s: (e + 1) * C_s], C_s))
        m.update(shared_w)
        in_maps.append(m)

    global _active_build_key
    _active_build_key = _build_key(C_r, C_s)
    try:
        res = run_bass_kernel_spmd(nc, in_maps, list(range(NCORES))).results
    finally:
        _active_build_key = None

    # ---- host combine (gates already applied on-chip)
    out = np.zeros((T, H), np.float32)
    for e in range(E):
        rows = idx_e[e]
        out[rows] += res[e]["y_r"][: len(rows)]
        lo = e * C_s
        hi = min(T, (e + 1) * C_s)
        if lo < hi:
            out[lo:hi] += res[e]["y_s"][: hi - lo]
    return out.reshape(orig_shape).astype(np.float32)
